# revision 1
# baseline (speedup 1.0000x reference)
"""Cross-modal attention TRN2 kernel.

Problem: B=4, N=2048, IN_DIM=DIM=1024, HEADS=8, D_HEAD=128, scale=DIM**-0.5.
  q = x_a @ W_q.T ; k,v = split(x_b @ W_kv.T) ; per-head softmax(q k^T/32) v ;
  out = merge_heads @ W_out.T + b_out

Sharding over 8 cores: core c -> batch b=c//2, head-half hh=c%2 (4 heads,
512 of DIM).  W_q/W_kv column-sharded, W_out row-sharded (Megatron); each
core emits a partial output projection y_cT = (W_out[:, slice] @ O_half)
of shape [DIM, N]; host sums the two head-half partials per batch, adds
b_out, transposes back.

Device layout: everything transposed ([feature, token]) so all matmuls
contract over the partition dim.  Host feeds x^T and W^T (cheap numpy
prep); device does:
  phase 1: Q^T = WqT.T @ xaT, K^T likewise, V (natural [j, dv])
  phase 2: per (head, 1024-token block): dots^T = K_tile^T.T... i.e.
           s^T[j,i] = sum_d K^T[d,j] Q^T[d,i]; exp on ACT (no max
           subtraction -- |s*scale| < ~1 by construction of the problem
           scale); PV and a ones-row matmul (denominator) accumulate over
           j-tiles in PSUM; normalize with reciprocal broadcast.
  phase 3: y^T = WoT.T @ O^T, DMA PSUM->DRAM.
All matmuls run as float32r (full PE rate at N=512).
"""

import numpy as np

B, N, IN_DIM, DIM, HEADS = 4, 2048, 1024, 1024, 8
D_HEAD = DIM // HEADS          # 128
SCALE = DIM ** -0.5            # 1/32
NCORES = 8
HH = HEADS // 2                # 4 heads per core
DVC = HH * D_HEAD              # 512 dv per core
P = 128
KT = IN_DIM // P               # 8 contraction tiles
NJT = N // P                   # 16 j tiles
NIB = N // 512                 # 4 i-blocks of 512
IB2 = N // 1024                # 2 i-blocks of 1024

_TRACE = False
REPS = 1
LAST_EXEC_NS = None
LAST_RESULTS = None
_nc_cache = []


def _build_nc(reps=1):
    import concourse.tile as tile
    from concourse import bacc, mybir

    f32 = mybir.dt.float32
    f32r = mybir.dt.float32r
    Exp = mybir.ActivationFunctionType.Exp

    nc = bacc.Bacc("TRN2", debug=False, num_devices=NCORES)

    xaT = nc.dram_tensor("xaT", [IN_DIM, N], f32r, kind="ExternalInput").ap()
    xbT = nc.dram_tensor("xbT", [IN_DIM, N], f32r, kind="ExternalInput").ap()
    wqT = nc.dram_tensor("wqT", [IN_DIM, DVC], f32r, kind="ExternalInput").ap()
    wkT = nc.dram_tensor("wkT", [IN_DIM, DVC], f32r, kind="ExternalInput").ap()
    wvT = nc.dram_tensor("wvT", [IN_DIM, DVC], f32r, kind="ExternalInput").ap()
    woT = nc.dram_tensor("woT", [DVC, DIM], f32r, kind="ExternalInput").ap()
    ones_d = nc.dram_tensor("ones", [P, 1], f32r, kind="ExternalInput").ap()
    yT = nc.dram_tensor("yT", [DIM, N], f32, kind="ExternalOutput").ap()

    with tile.TileContext(nc) as tc:
      for _rep in range(reps):
        with tc.tile_pool(name="persist", bufs=1) as persist:
            qT_sb = persist.tile([P, HH, N], f32r)      # [d%128, head, i]
            kT_sb = persist.tile([P, HH, N], f32r)      # [d%128, head, j]
            v_sb = persist.tile([P, NJT, DVC], f32r)    # [j%128, jt, dv]
            oT_ts = [[persist.tile([P, 1024], f32r, tag=f"o{h}_{bb}",
                                   name=f"o{h}_{bb}")
                      for bb in range(IB2)] for h in range(HH)]
            ones_sb = persist.tile([P, 1], f32r)
            nc.sync.dma_start(out=ones_sb, in_=ones_d)

            # ---------------- phase 1: projections ----------------
            BW = 256  # streaming block width (>=256 keeps f32r full rate)
            NB = N // BW
            with tc.tile_pool(name="wpool", bufs=1) as wpool, \
                 tc.tile_pool(name="xblk", bufs=3) as xblk, \
                 tc.tile_pool(name="psum1", bufs=4, space="PSUM") as psum1:
                wq_ts = [wpool.tile([P, DVC], f32r, tag=f"wq{kt}", name=f"wq{kt}")
                         for kt in range(KT)]
                wk_ts = [wpool.tile([P, DVC], f32r, tag=f"wk{kt}", name=f"wk{kt}")
                         for kt in range(KT)]
                wv_ts = [wpool.tile([P, DVC], f32r, tag=f"wv{kt}", name=f"wv{kt}")
                         for kt in range(KT)]

                for ib in range(NB):
                    xa_blk = xblk.tile([P, KT, BW], f32r, tag="xblk")
                    nc.sync.dma_start(
                        out=xa_blk,
                        in_=xaT[:, ib * BW:(ib + 1) * BW]
                        .rearrange("(kt p) i -> p kt i", p=P))
                    if ib == 0:
                        # after the first x block so the first matmul's
                        # operands land earliest in DMA queue order
                        for kt in range(KT):
                            nc.sync.dma_start(
                                out=wq_ts[kt],
                                in_=wqT[kt * P:(kt + 1) * P, :])
                    for dt in range(HH):
                        ps = psum1.tile([P, BW], f32, tag="ps1")
                        for kt in range(KT):
                            nc.tensor.matmul(
                                ps,
                                wq_ts[kt][:, dt * P:(dt + 1) * P],
                                xa_blk[:, kt, :],
                                start=(kt == 0), stop=(kt == KT - 1))
                        nc.vector.tensor_copy(
                            qT_sb[:, dt, ib * BW:(ib + 1) * BW], ps)

                for jb in range(NB):
                    xb_blk = xblk.tile([P, KT, BW], f32r, tag="xblk")
                    nc.sync.dma_start(
                        out=xb_blk,
                        in_=xbT[:, jb * BW:(jb + 1) * BW]
                        .rearrange("(kt p) i -> p kt i", p=P))
                    if jb == 0:
                        for kt in range(KT):
                            nc.sync.dma_start(
                                out=wk_ts[kt],
                                in_=wkT[kt * P:(kt + 1) * P, :])
                            nc.sync.dma_start(
                                out=wv_ts[kt],
                                in_=wvT[kt * P:(kt + 1) * P, :])
                    for dt in range(HH):
                        ps = psum1.tile([P, BW], f32, tag="ps1")
                        for kt in range(KT):
                            nc.tensor.matmul(
                                ps,
                                wk_ts[kt][:, dt * P:(dt + 1) * P],
                                xb_blk[:, kt, :],
                                start=(kt == 0), stop=(kt == KT - 1))
                        nc.vector.tensor_copy(
                            kT_sb[:, dt, jb * BW:(jb + 1) * BW], ps)
                    for j2 in range(BW // P):
                        jt = jb * (BW // P) + j2
                        ps = psum1.tile([P, DVC], f32, tag="psv")
                        for kt in range(KT):
                            nc.tensor.matmul(
                                ps,
                                xb_blk[:, kt, j2 * P:(j2 + 1) * P],
                                wv_ts[kt],
                                start=(kt == 0), stop=(kt == KT - 1))
                        nc.vector.tensor_copy(v_sb[:, jt, :], ps)

            # ---------------- phase 2: attention ----------------
            with tc.tile_pool(name="expp", bufs=8) as expp, \
                 tc.tile_pool(name="bcp", bufs=2) as bcp, \
                 tc.tile_pool(name="rcp", bufs=2) as rcp, \
                 tc.tile_pool(name="dotsp", bufs=2, space="PSUM") as dotsp, \
                 tc.tile_pool(name="avp", bufs=1, space="PSUM") as avp, \
                 tc.tile_pool(name="denp", bufs=1, space="PSUM") as denp:
                LAG = 2   # PV/ones trail dots/exp by 2 j-tiles so the PE
                          # never waits on the ACT exp of the current tile
                for ib in range(IB2):
                    for h in range(HH):
                        i0 = ib * 1024
                        po = avp.tile([P, 1024], f32)
                        pd = denp.tile([1, 1024], f32)
                        ets = {}
                        for jt in range(NJT + LAG):
                            if jt < NJT:
                                ps = dotsp.tile([P, 1024], f32, tag="ps")
                                k_l = kT_sb[:, h, jt * P:(jt + 1) * P]
                                for hf in range(2):
                                    nc.tensor.matmul(
                                        ps[:, hf * 512:(hf + 1) * 512],
                                        k_l,
                                        qT_sb[:, h,
                                              i0 + hf * 512:i0 + (hf + 1) * 512],
                                        start=True, stop=True)
                                et = expp.tile([P, 1024], f32r, tag="exp")
                                nc.scalar.activation(et, ps, Exp, scale=SCALE)
                                ets[jt] = et
                            if jt >= LAG:
                                jd = jt - LAG
                                et = ets.pop(jd)
                                v_l = v_sb[:, jd, h * P:(h + 1) * P]
                                for hf in range(2):
                                    sl = slice(hf * 512, (hf + 1) * 512)
                                    nc.tensor.matmul(
                                        po[:, sl], v_l, et[:, sl],
                                        start=(jd == 0), stop=(jd == NJT - 1))
                                    nc.tensor.matmul(
                                        pd[:, sl], ones_sb,
                                        et[:, sl],
                                        start=(jd == 0), stop=(jd == NJT - 1))
                        # drain the PV accumulator to SBUF right away so the
                        # PSUM bank frees for the next block; normalize there.
                        osl = oT_ts[h][ib]
                        nc.vector.tensor_copy(osl, po)
                        rc = rcp.tile([1, 1024], f32, tag="rc")
                        nc.vector.reciprocal(rc, pd)
                        bc = bcp.tile([P, 1024], f32, tag="bc")
                        nc.gpsimd.partition_broadcast(bc, rc)
                        nc.vector.tensor_mul(osl, osl, bc)

                # ---------------- phase 3: output projection ----------------
                # y-psum tiles share the dots pool slots (tag "ps"), which
                # free as the exp of the final j-tiles completes -- a fresh
                # PSUM pool would wait on the whole attention stack instead.
                with tc.tile_pool(name="wop", bufs=1) as wop, \
                     tc.tile_pool(name="ystage", bufs=4) as ystage:
                    wo_sb = wop.tile([P, HH, DIM], f32r)
                    nc.sync.dma_start(
                        out=wo_sb, in_=woT.rearrange("(dt p) e -> p dt e", p=P))
                    for ib in range(NIB):
                        bb, half = divmod(ib, 2)
                        for e8 in range(DIM // P):
                            ps = dotsp.tile([P, 512], f32, tag="ps")
                            for dt in range(HH):
                                nc.tensor.matmul(
                                    ps,
                                    wo_sb[:, dt, e8 * P:(e8 + 1) * P],
                                    oT_ts[dt][bb][:, half * 512:(half + 1) * 512],
                                    start=(dt == 0), stop=(dt == HH - 1))
                            ys = ystage.tile([P, 512], f32, tag="ys")
                            if ib % 2 == 0:
                                nc.vector.tensor_copy(ys, ps)
                            else:
                                nc.scalar.copy(ys, ps)
                            nc.sync.dma_start(
                                out=yT[e8 * P:(e8 + 1) * P,
                                       ib * 512:(ib + 1) * 512],
                                in_=ys)

    nc.compile()
    return nc


_nc_by_reps = {}


def _get_nc(reps=1):
    if reps not in _nc_by_reps:
        _nc_by_reps[reps] = _build_nc(reps)
    return _nc_by_reps[reps]


def kernel(x_a, x_b, W_q, W_kv, W_out, b_out):
    global LAST_EXEC_NS, LAST_RESULTS
    from concourse import bass_utils

    x_a = np.asarray(x_a, dtype=np.float32)
    x_b = np.asarray(x_b, dtype=np.float32)
    W_q = np.asarray(W_q, dtype=np.float32)
    W_kv = np.asarray(W_kv, dtype=np.float32)
    W_out = np.asarray(W_out, dtype=np.float32)
    b_out = np.asarray(b_out, dtype=np.float32)

    nc = _get_nc(REPS)

    xaT = [np.ascontiguousarray(x_a[b].T) for b in range(B)]
    xbT = [np.ascontiguousarray(x_b[b].T) for b in range(B)]
    in_maps = []
    for c in range(NCORES):
        b, hh = divmod(c, 2)
        hs = hh * DVC
        in_maps.append({
            "xaT": xaT[b],
            "xbT": xbT[b],
            "wqT": np.ascontiguousarray(W_q[hs:hs + DVC].T),
            "wkT": np.ascontiguousarray(W_kv[hs:hs + DVC].T),
            "wvT": np.ascontiguousarray(W_kv[DIM + hs:DIM + hs + DVC].T),
            "woT": np.ascontiguousarray(W_out[:, hs:hs + DVC].T),
            "ones": np.ones((P, 1), dtype=np.float32),
        })

    res = bass_utils.run_bass_kernel_spmd(
        nc, in_maps, core_ids=list(range(NCORES)), trace=_TRACE)
    LAST_EXEC_NS = res.exec_time_ns
    LAST_RESULTS = res

    out = np.empty((B, N, DIM), dtype=np.float32)
    for b in range(B):
        acc = res.results[2 * b]["yT"] + res.results[2 * b + 1]["yT"]
        out[b] = acc.T + b_out
    return out


def _make_in_maps(x_a, x_b, W_q, W_kv, W_out):
    xaT = [np.ascontiguousarray(x_a[b].T) for b in range(B)]
    xbT = [np.ascontiguousarray(x_b[b].T) for b in range(B)]
    in_maps = []
    for c in range(NCORES):
        b, hh = divmod(c, 2)
        hs = hh * DVC
        in_maps.append({
            "xaT": xaT[b],
            "xbT": xbT[b],
            "wqT": np.ascontiguousarray(W_q[hs:hs + DVC].T),
            "wkT": np.ascontiguousarray(W_kv[hs:hs + DVC].T),
            "wvT": np.ascontiguousarray(W_kv[DIM + hs:DIM + hs + DVC].T),
            "woT": np.ascontiguousarray(W_out[:, hs:hs + DVC].T),
            "ones": np.ones((P, 1), dtype=np.float32),
        })
    return in_maps


def bench(inputs, reps_pair=(1, 9), iters=5):
    """Measure on-device time per kernel body via rep-delta wall timing."""
    import time
    from concourse import bass_utils
    ins = {k: np.asarray(v, dtype=np.float32) for k, v in inputs.items()
           if k != "b_out"}
    in_maps = _make_in_maps(ins["x_a"], ins["x_b"], ins["W_q"], ins["W_kv"],
                            ins["W_out"])
    walls = {}
    for reps in reps_pair:
        nc = _get_nc(reps)
        # warm-up (compile+cache)
        bass_utils.run_bass_kernel_spmd(nc, in_maps, core_ids=list(range(NCORES)))
        ts = []
        for _ in range(iters):
            t0 = time.perf_counter()
            bass_utils.run_bass_kernel_spmd(nc, in_maps,
                                            core_ids=list(range(NCORES)))
            ts.append(time.perf_counter() - t0)
        walls[reps] = min(ts)
        print(f"reps={reps}: wall min={walls[reps]*1e3:.2f} ms  all={[f'{t*1e3:.1f}' for t in ts]}")
    r0, r1 = reps_pair
    ns = (walls[r1] - walls[r0]) / (r1 - r0) * 1e9
    print(f"per-body device time: {ns:.0f} ns")
    return ns



# revision 2
# speedup vs baseline: 1.2935x; 1.2935x over previous
"""Cross-modal attention TRN2 kernel (bf16 on-chip).

Problem: B=4, N=2048, IN_DIM=DIM=1024, HEADS=8, D_HEAD=128, scale=DIM**-0.5.
  q = x_a @ W_q.T ; k,v = split(x_b @ W_kv.T) ; per-head softmax(q k^T/32) v ;
  out = merge_heads @ W_out.T + b_out

Sharding over 8 cores: core c -> batch b=c//2, head-half hh=c%2 (4 heads,
512 of DIM).  W_q/W_kv column-sharded, W_out row-sharded (Megatron); each
core emits a partial output projection y_cT = (W_out[:, slice] @ O_half)
of shape [DIM, N] in bf16; host sums the two head-half partials per batch
in f32, adds b_out, transposes back.

All on-chip operands are bf16 (PSUM accumulation stays f32):
 - halves HBM traffic (12 MB in / 4 MB out per core),
 - enables the compiler's fast-weight-load path so LDWEIGHTS (~53 ns)
   hides fully under the 512-row matmuls (~216 ns) -- fp32r paid ~204 ns
   per load which stretched the matmul cadence to ~253 ns.

Device layout: everything transposed ([feature, token]) so all matmuls
contract over the partition dim.
  phase 1: Q^T = WqT.T @ xaT, K^T likewise, V (natural [j, dv]), 512-wide
           token blocks, outputs copied to SBUF as bf16.
  phase 2: per (head, 1024-token block): s^T[j,i] on PE; exp on ACT (no
           max subtraction -- |s*scale| < ~1 by construction); PV and a
           ones-row matmul (denominator) accumulate over j-tiles in PSUM.
           Normalize drain: ACT copies the denominator row out of PSUM
           (frees the bank fast), DVE reciprocal_approx_fast (the exact
           reciprocal is ~6.6 cyc/elem and stalled the PE ~5 us per
           block), GpSimd broadcast, DVE multiply.
  phase 3: y^T = WoT.T @ O^T, staged to SBUF bf16, DMA to DRAM.
"""

import numpy as np

B, N, IN_DIM, DIM, HEADS = 4, 2048, 1024, 1024, 8
D_HEAD = DIM // HEADS          # 128
SCALE = DIM ** -0.5            # 1/32
NCORES = 8
HH = HEADS // 2                # 4 heads per core
DVC = HH * D_HEAD              # 512 dv per core
P = 128
KT = IN_DIM // P               # 8 contraction tiles
NJT = N // P                   # 16 j tiles
NIB = N // 512                 # 4 i-blocks of 512 (phase 3)
IB2 = N // 1024                # 2 i-blocks of 1024 (phase 2)

_TRACE = False
REPS = 1
LAST_EXEC_NS = None
LAST_RESULTS = None


def _build_nc(reps=1):
    import concourse.tile as tile
    from concourse import bacc, mybir

    f32 = mybir.dt.float32
    bf16 = mybir.dt.bfloat16
    Exp = mybir.ActivationFunctionType.Exp

    nc = bacc.Bacc("TRN2", debug=False, num_devices=NCORES)

    xaT = nc.dram_tensor("xaT", [IN_DIM, N], bf16, kind="ExternalInput").ap()
    xbT = nc.dram_tensor("xbT", [IN_DIM, N], bf16, kind="ExternalInput").ap()
    wqT = nc.dram_tensor("wqT", [IN_DIM, DVC], bf16, kind="ExternalInput").ap()
    wkT = nc.dram_tensor("wkT", [IN_DIM, DVC], bf16, kind="ExternalInput").ap()
    wvT = nc.dram_tensor("wvT", [IN_DIM, DVC], bf16, kind="ExternalInput").ap()
    woT = nc.dram_tensor("woT", [DVC, DIM], bf16, kind="ExternalInput").ap()
    ones_d = nc.dram_tensor("ones", [P, 1], bf16, kind="ExternalInput").ap()
    yT = nc.dram_tensor("yT", [DIM, N], bf16, kind="ExternalOutput").ap()

    with tile.TileContext(nc) as tc:
      for _rep in range(reps):
        with tc.tile_pool(name="persist", bufs=1) as persist:
            qT_sb = persist.tile([P, HH, N], bf16)      # [d%128, head, i]
            kT_sb = persist.tile([P, HH, N], bf16)      # [d%128, head, j]
            v_sb = persist.tile([P, NJT, DVC], bf16)    # [j%128, jt, dv]
            oT_ts = [[persist.tile([P, 1024], bf16, tag=f"o{h}_{bb}",
                                   name=f"o{h}_{bb}")
                      for bb in range(IB2)] for h in range(HH)]
            ones_sb = persist.tile([P, 1], bf16)
            nc.sync.dma_start(out=ones_sb, in_=ones_d)

            # ---------------- phase 1: projections ----------------
            BW = 512  # streaming block width (1 PSUM bank of f32)
            NB = N // BW
            with tc.tile_pool(name="wpool", bufs=1) as wpool, \
                 tc.tile_pool(name="xblk", bufs=3) as xblk, \
                 tc.tile_pool(name="psum1", bufs=4, space="PSUM") as psum1:
                wq_ts = [wpool.tile([P, DVC], bf16, tag=f"wq{kt}", name=f"wq{kt}")
                         for kt in range(KT)]
                wk_ts = [wpool.tile([P, DVC], bf16, tag=f"wk{kt}", name=f"wk{kt}")
                         for kt in range(KT)]
                wv_ts = [wpool.tile([P, DVC], bf16, tag=f"wv{kt}", name=f"wv{kt}")
                         for kt in range(KT)]

                for ib in range(NB):
                    xa_blk = xblk.tile([P, KT, BW], bf16, tag="xblk")
                    nc.sync.dma_start(
                        out=xa_blk,
                        in_=xaT[:, ib * BW:(ib + 1) * BW]
                        .rearrange("(kt p) i -> p kt i", p=P))
                    if ib == 0:
                        # after the first x block so the first matmul's
                        # operands land earliest in DMA queue order
                        for kt in range(KT):
                            nc.sync.dma_start(
                                out=wq_ts[kt],
                                in_=wqT[kt * P:(kt + 1) * P, :])
                    for dt in range(HH):
                        ps = psum1.tile([P, BW], f32, tag="ps1")
                        for kt in range(KT):
                            nc.tensor.matmul(
                                ps,
                                wq_ts[kt][:, dt * P:(dt + 1) * P],
                                xa_blk[:, kt, :],
                                start=(kt == 0), stop=(kt == KT - 1))
                        nc.vector.tensor_copy(
                            qT_sb[:, dt, ib * BW:(ib + 1) * BW], ps)

                for jb in range(NB):
                    xb_blk = xblk.tile([P, KT, BW], bf16, tag="xblk")
                    nc.sync.dma_start(
                        out=xb_blk,
                        in_=xbT[:, jb * BW:(jb + 1) * BW]
                        .rearrange("(kt p) i -> p kt i", p=P))
                    if jb == 0:
                        for kt in range(KT):
                            nc.sync.dma_start(
                                out=wk_ts[kt],
                                in_=wkT[kt * P:(kt + 1) * P, :])
                            nc.sync.dma_start(
                                out=wv_ts[kt],
                                in_=wvT[kt * P:(kt + 1) * P, :])
                    for dt in range(HH):
                        ps = psum1.tile([P, BW], f32, tag="ps1")
                        for kt in range(KT):
                            nc.tensor.matmul(
                                ps,
                                wk_ts[kt][:, dt * P:(dt + 1) * P],
                                xb_blk[:, kt, :],
                                start=(kt == 0), stop=(kt == KT - 1))
                        nc.vector.tensor_copy(
                            kT_sb[:, dt, jb * BW:(jb + 1) * BW], ps)
                    for j2 in range(BW // P):
                        jt = jb * (BW // P) + j2
                        ps = psum1.tile([P, DVC], f32, tag="psv")
                        for kt in range(KT):
                            nc.tensor.matmul(
                                ps,
                                xb_blk[:, kt, j2 * P:(j2 + 1) * P],
                                wv_ts[kt],
                                start=(kt == 0), stop=(kt == KT - 1))
                        nc.vector.tensor_copy(v_sb[:, jt, :], ps)

            # ---------------- phase 2: attention ----------------
            with tc.tile_pool(name="expp", bufs=6) as expp, \
                 tc.tile_pool(name="drainp", bufs=2) as drainp, \
                 tc.tile_pool(name="dotsp", bufs=2, space="PSUM") as dotsp, \
                 tc.tile_pool(name="avp", bufs=1, space="PSUM") as avp, \
                 tc.tile_pool(name="denp", bufs=1, space="PSUM") as denp:
                LAG = 2   # PV/ones trail dots/exp by 2 j-tiles so the PE
                          # never waits on the ACT exp of the current tile
                for ib in range(IB2):
                    for h in range(HH):
                        i0 = ib * 1024
                        po = avp.tile([P, 1024], f32)
                        pd = denp.tile([1, 1024], f32)
                        ets = {}
                        for jt in range(NJT + LAG):
                            if jt < NJT:
                                ps = dotsp.tile([P, 1024], f32, tag="ps")
                                k_l = kT_sb[:, h, jt * P:(jt + 1) * P]
                                for hf in range(2):
                                    nc.tensor.matmul(
                                        ps[:, hf * 512:(hf + 1) * 512],
                                        k_l,
                                        qT_sb[:, h,
                                              i0 + hf * 512:i0 + (hf + 1) * 512],
                                        start=True, stop=True)
                                et = expp.tile([P, 1024], bf16, tag="exp")
                                nc.scalar.activation(et, ps, Exp, scale=SCALE)
                                ets[jt] = et
                            if jt >= LAG:
                                jd = jt - LAG
                                et = ets.pop(jd)
                                v_l = v_sb[:, jd, h * P:(h + 1) * P]
                                for hf in range(2):
                                    sl = slice(hf * 512, (hf + 1) * 512)
                                    nc.tensor.matmul(
                                        po[:, sl], v_l, et[:, sl],
                                        start=(jd == 0), stop=(jd == NJT - 1))
                                for hf in range(2):
                                    sl = slice(hf * 512, (hf + 1) * 512)
                                    nc.tensor.matmul(
                                        pd[:, sl], ones_sb,
                                        et[:, sl],
                                        start=(jd == 0), stop=(jd == NJT - 1))
                        # Drain: free the PV and denominator PSUM banks as
                        # fast as possible (next block's accumulations wait
                        # on them), then normalize off the critical path.
                        osl = oT_ts[h][ib]
                        nc.vector.tensor_copy(osl, po)
                        dsb = drainp.tile([1, 1024], f32, tag="den")
                        nc.scalar.copy(dsb, pd)
                        rcf = drainp.tile([1, 1024], f32, tag="rcf")
                        nc.vector.reciprocal_approx_fast(rcf, dsb)
                        rcb = drainp.tile([1, 1024], bf16, tag="rcb")
                        nc.vector.tensor_copy(rcb, rcf)
                        bc = drainp.tile([P, 1024], bf16, tag="bc")
                        nc.gpsimd.partition_broadcast(bc, rcb)
                        nc.vector.tensor_mul(osl, osl, bc)

                # ---------------- phase 3: output projection ----------------
                # y-psum tiles share the dots pool slots (tag "ps"), which
                # free as the exp of the final j-tiles completes -- a fresh
                # PSUM pool would wait on the whole attention stack instead.
                with tc.tile_pool(name="wop", bufs=1) as wop, \
                     tc.tile_pool(name="ystage", bufs=4) as ystage:
                    wo_sb = wop.tile([P, HH, DIM], bf16)
                    nc.sync.dma_start(
                        out=wo_sb, in_=woT.rearrange("(dt p) e -> p dt e", p=P))
                    for ib in range(NIB):
                        bb, half = divmod(ib, 2)
                        for e8 in range(DIM // P):
                            ps = dotsp.tile([P, 512], f32, tag="ps")
                            for dt in range(HH):
                                nc.tensor.matmul(
                                    ps,
                                    wo_sb[:, dt, e8 * P:(e8 + 1) * P],
                                    oT_ts[dt][bb][:, half * 512:(half + 1) * 512],
                                    start=(dt == 0), stop=(dt == HH - 1))
                            ys = ystage.tile([P, 512], bf16, tag="ys")
                            if ib % 2 == 0:
                                nc.vector.tensor_copy(ys, ps)
                            else:
                                nc.scalar.copy(ys, ps)
                            nc.sync.dma_start(
                                out=yT[e8 * P:(e8 + 1) * P,
                                       ib * 512:(ib + 1) * 512],
                                in_=ys)

    nc.compile()
    return nc


_nc_by_reps = {}


def _get_nc(reps=1):
    if reps not in _nc_by_reps:
        _nc_by_reps[reps] = _build_nc(reps)
    return _nc_by_reps[reps]


def _make_in_maps(x_a, x_b, W_q, W_kv, W_out):
    from concourse import mybir
    BF = mybir.dt.np(mybir.dt.bfloat16)
    xaT = [np.ascontiguousarray(x_a[b].T).astype(BF) for b in range(B)]
    xbT = [np.ascontiguousarray(x_b[b].T).astype(BF) for b in range(B)]
    in_maps = []
    for c in range(NCORES):
        b, hh = divmod(c, 2)
        hs = hh * DVC
        in_maps.append({
            "xaT": xaT[b],
            "xbT": xbT[b],
            "wqT": np.ascontiguousarray(W_q[hs:hs + DVC].T).astype(BF),
            "wkT": np.ascontiguousarray(W_kv[hs:hs + DVC].T).astype(BF),
            "wvT": np.ascontiguousarray(
                W_kv[DIM + hs:DIM + hs + DVC].T).astype(BF),
            "woT": np.ascontiguousarray(W_out[:, hs:hs + DVC].T).astype(BF),
            "ones": np.ones((P, 1), dtype=BF),
        })
    return in_maps


def kernel(x_a, x_b, W_q, W_kv, W_out, b_out):
    global LAST_EXEC_NS, LAST_RESULTS
    from concourse import bass_utils

    x_a = np.asarray(x_a, dtype=np.float32)
    x_b = np.asarray(x_b, dtype=np.float32)
    W_q = np.asarray(W_q, dtype=np.float32)
    W_kv = np.asarray(W_kv, dtype=np.float32)
    W_out = np.asarray(W_out, dtype=np.float32)
    b_out = np.asarray(b_out, dtype=np.float32)

    nc = _get_nc(REPS)
    in_maps = _make_in_maps(x_a, x_b, W_q, W_kv, W_out)

    res = bass_utils.run_bass_kernel_spmd(
        nc, in_maps, core_ids=list(range(NCORES)), trace=_TRACE)
    LAST_EXEC_NS = res.exec_time_ns
    LAST_RESULTS = res

    out = np.empty((B, N, DIM), dtype=np.float32)
    for b in range(B):
        acc = (res.results[2 * b]["yT"].astype(np.float32)
               + res.results[2 * b + 1]["yT"].astype(np.float32))
        out[b] = acc.T + b_out
    return out


def bench(inputs, reps_pair=(1, 9), iters=5):
    """Measure on-device time per kernel body via rep-delta wall timing."""
    import time
    from concourse import bass_utils
    ins = {k: np.asarray(v, dtype=np.float32) for k, v in inputs.items()
           if k != "b_out"}
    in_maps = _make_in_maps(ins["x_a"], ins["x_b"], ins["W_q"], ins["W_kv"],
                            ins["W_out"])
    walls = {}
    for reps in reps_pair:
        nc = _get_nc(reps)
        # warm-up (compile+cache)
        bass_utils.run_bass_kernel_spmd(nc, in_maps, core_ids=list(range(NCORES)))
        ts = []
        for _ in range(iters):
            t0 = time.perf_counter()
            bass_utils.run_bass_kernel_spmd(nc, in_maps,
                                            core_ids=list(range(NCORES)))
            ts.append(time.perf_counter() - t0)
        walls[reps] = min(ts)
        print(f"reps={reps}: wall min={walls[reps]*1e3:.2f} ms  all={[f'{t*1e3:.1f}' for t in ts]}")
    r0, r1 = reps_pair
    ns = (walls[r1] - walls[r0]) / (r1 - r0) * 1e9
    print(f"per-body device time: {ns:.0f} ns")
    return ns


# revision 6
# speedup vs baseline: 1.5450x; 1.1944x over previous
"""Cross-modal attention TRN2 kernel (bf16 on-chip).

Problem: B=4, N=2048, IN_DIM=DIM=1024, HEADS=8, D_HEAD=128, scale=DIM**-0.5.
  q = x_a @ W_q.T ; k,v = split(x_b @ W_kv.T) ; per-head softmax(q k^T/32) v ;
  out = merge_heads @ W_out.T + b_out

Sharding over 8 cores: core c -> batch b=c//2, head-half hh=c%2 (4 heads,
512 of DIM).  W_q/W_kv column-sharded, W_out row-sharded (Megatron); each
core emits a partial output projection y_cT = (W_out[:, slice] @ O_half)
of shape [DIM, N] in bf16; host sums the two head-half partials per batch
in f32, adds b_out, transposes back.

All on-chip operands are bf16 (PSUM accumulation stays f32):
 - halves HBM traffic (12 MB in / 4 MB out per core),
 - enables the compiler's fast-weight-load path so LDWEIGHTS (~53 ns)
   hides fully under the 512-row matmuls (~216 ns) -- fp32r paid ~204 ns
   per load which stretched the matmul cadence to ~253 ns.

Device layout: everything transposed ([feature, token]) so all matmuls
contract over the partition dim.
  phase 1: Q^T = WqT.T @ xaT, K^T likewise, V (natural [j, dv]), 512-wide
           token blocks, outputs copied to SBUF as bf16.
  phase 2: per (head, 1024-token block): s^T[j,i] on PE; exp on ACT (no
           max subtraction -- |s*scale| < ~1 by construction); PV and a
           ones-row matmul (denominator) accumulate over j-tiles in PSUM.
           Normalize drain: ACT copies the denominator row out of PSUM
           (frees the bank fast), DVE reciprocal_approx_fast (the exact
           reciprocal is ~6.6 cyc/elem and stalled the PE ~5 us per
           block), GpSimd broadcast, DVE multiply.
  phase 3: y^T = WoT.T @ O^T, staged to SBUF bf16, DMA to DRAM.
"""

import numpy as np

B, N, IN_DIM, DIM, HEADS = 4, 2048, 1024, 1024, 8
D_HEAD = DIM // HEADS          # 128
SCALE = DIM ** -0.5            # 1/32
NCORES = 8
HH = HEADS // 2                # 4 heads per core
DVC = HH * D_HEAD              # 512 dv per core
P = 128
KT = IN_DIM // P               # 8 contraction tiles
NJT = N // P                   # 16 j tiles
NIB = N // 512                 # 4 i-blocks of 512 (phase 3)
IB2 = N // 1024                # 2 i-blocks of 1024 (phase 2)

_TRACE = False
REPS = 1
LAST_EXEC_NS = None
LAST_RESULTS = None


def _build_nc(reps=1):
    import concourse.tile as tile
    from concourse import bacc, mybir

    f32 = mybir.dt.float32
    bf16 = mybir.dt.bfloat16
    fp8 = mybir.dt.float8e4
    DR = mybir.MatmulPerfMode.DoubleRow
    Exp = mybir.ActivationFunctionType.Exp

    nc = bacc.Bacc("TRN2", debug=False, num_devices=NCORES)

    # Q/K projections run in fp8e4m3 DoubleRow (2 MACs/cell, LDW-bound at
    # ~2x the bf16 rate).  W_q/W_k are pre-scaled by 16 host-side so their
    # values sit in e4m3's normal range; the extra 256x on the dots is
    # folded into the exp scale.  x_b is shipped twice: fp8 for the K
    # matmuls, bf16 for V (fp8 V would put ~2.5% error straight on the
    # output; Q/K errors wash out through the softmax's tiny 1/32 scale).
    xa8 = nc.dram_tensor("xa8", [IN_DIM, N], fp8, kind="ExternalInput").ap()
    xb8 = nc.dram_tensor("xb8", [IN_DIM, N], fp8, kind="ExternalInput").ap()
    xbT = nc.dram_tensor("xbT", [IN_DIM, N], bf16, kind="ExternalInput").ap()
    wq8 = nc.dram_tensor("wq8", [IN_DIM, DVC], fp8, kind="ExternalInput").ap()
    wk8 = nc.dram_tensor("wk8", [IN_DIM, DVC], fp8, kind="ExternalInput").ap()
    wvT = nc.dram_tensor("wvT", [IN_DIM, DVC], bf16, kind="ExternalInput").ap()
    woT = nc.dram_tensor("woT", [DVC, DIM], bf16, kind="ExternalInput").ap()
    ones_d = nc.dram_tensor("ones", [P, 1], bf16, kind="ExternalInput").ap()
    yT = nc.dram_tensor("yT", [DIM, N], bf16, kind="ExternalOutput").ap()
    SCALE_EXP = SCALE / 256.0  # W_q, W_k each pre-scaled by 16

    with tile.TileContext(nc) as tc:
      for _rep in range(reps):
        with tc.tile_pool(name="persist", bufs=1) as persist:
            qT_sb = persist.tile([P, HH, N], bf16)      # [d%128, head, i]
            kT_sb = persist.tile([P, HH, N], bf16)      # [d%128, head, j]
            v_sb = persist.tile([P, NJT, DVC], bf16)    # [j%128, jt, dv]
            oT_ts = [[persist.tile([P, 1024], bf16, tag=f"o{h}_{bb}",
                                   name=f"o{h}_{bb}")
                      for bb in range(IB2)] for h in range(HH)]
            ones_sb = persist.tile([P, 1], bf16)
            nc.sync.dma_start(out=ones_sb, in_=ones_d)

            # ---------------- phase 1: projections ----------------
            BW = 512  # streaming block width (1 PSUM bank of f32)
            NB = N // BW
            with tc.tile_pool(name="wpool", bufs=1) as wpool, \
                 tc.tile_pool(name="xblk", bufs=3) as xblk, \
                 tc.tile_pool(name="psum1", bufs=4, space="PSUM") as psum1:
                wq_t = wpool.tile([P, KT, DVC], fp8, tag="wq", name="wq")
                wk_t = wpool.tile([P, KT, DVC], fp8, tag="wk", name="wk")
                wv_t = wpool.tile([P, KT, DVC], bf16, tag="wv", name="wv")

                for ib in range(NB):
                    xa_blk = xblk.tile([P, KT, BW], fp8, tag="xa")
                    nc.sync.dma_start(
                        out=xa_blk,
                        in_=xa8[:, ib * BW:(ib + 1) * BW]
                        .rearrange("(kt p) i -> p kt i", p=P))
                    if ib == 0:
                        # after the first x block so the first matmul's
                        # operands land earliest in DMA queue order
                        nc.sync.dma_start(
                            out=wq_t,
                            in_=wq8.rearrange("(kt p) d -> p kt d", p=P))
                    for dt in range(HH):
                        ps = psum1.tile([P, BW], f32, tag="ps1")
                        for kp in range(KT // 2):
                            nc.tensor.matmul(
                                ps,
                                wq_t[:, 2 * kp:2 * kp + 2, dt * P:(dt + 1) * P],
                                xa_blk[:, 2 * kp:2 * kp + 2, :],
                                start=(kp == 0), stop=(kp == KT // 2 - 1),
                                perf_mode=DR)
                        nc.vector.tensor_copy(
                            qT_sb[:, dt, ib * BW:(ib + 1) * BW], ps)

                for jb in range(NB):
                    xb_blk = xblk.tile([P, KT, BW], fp8, tag="xb8")
                    nc.sync.dma_start(
                        out=xb_blk,
                        in_=xb8[:, jb * BW:(jb + 1) * BW]
                        .rearrange("(kt p) i -> p kt i", p=P))
                    xbb_blk = xblk.tile([P, KT, BW], bf16, tag="xbb")
                    nc.sync.dma_start(
                        out=xbb_blk,
                        in_=xbT[:, jb * BW:(jb + 1) * BW]
                        .rearrange("(kt p) i -> p kt i", p=P))
                    if jb == 0:
                        nc.sync.dma_start(
                            out=wk_t,
                            in_=wk8.rearrange("(kt p) d -> p kt d", p=P))
                        nc.sync.dma_start(
                            out=wv_t,
                            in_=wvT.rearrange("(kt p) d -> p kt d", p=P))
                    for dt in range(HH):
                        ps = psum1.tile([P, BW], f32, tag="ps1")
                        for kp in range(KT // 2):
                            nc.tensor.matmul(
                                ps,
                                wk_t[:, 2 * kp:2 * kp + 2, dt * P:(dt + 1) * P],
                                xb_blk[:, 2 * kp:2 * kp + 2, :],
                                start=(kp == 0), stop=(kp == KT // 2 - 1),
                                perf_mode=DR)
                        nc.vector.tensor_copy(
                            kT_sb[:, dt, jb * BW:(jb + 1) * BW], ps)
                    for j2 in range(BW // P):
                        jt = jb * (BW // P) + j2
                        ps = psum1.tile([P, DVC], f32, tag="psv")
                        for kt in range(KT):
                            nc.tensor.matmul(
                                ps,
                                xbb_blk[:, kt, j2 * P:(j2 + 1) * P],
                                wv_t[:, kt, :],
                                start=(kt == 0), stop=(kt == KT - 1))
                        nc.vector.tensor_copy(v_sb[:, jt, :], ps)

            # ---------------- phase 2: attention ----------------
            with tc.tile_pool(name="expp", bufs=6) as expp, \
                 tc.tile_pool(name="sump", bufs=2) as sump, \
                 tc.tile_pool(name="drainp", bufs=2) as drainp, \
                 tc.tile_pool(name="dotsp", bufs=2, space="PSUM") as dotsp, \
                 tc.tile_pool(name="avp", bufs=1, space="PSUM") as avp, \
                 tc.tile_pool(name="denp", bufs=1, space="PSUM") as denp:
                LAG = 2   # PV/ones trail dots/exp by 2 j-tiles so the PE
                          # never waits on the ACT exp of the current tile
                # Denominator: j-tiles 0..11 are pre-summed in quads on the
                # DVE (3 adds) so one ones-matmul covers 4 tiles; the last 4
                # j-tiles go through per-tile ones-matmuls so the block tail
                # doesn't serialize behind the DVE adds.  bf16 quad-sums add
                # ~0.15% rms to the denominator -- well inside tolerance.
                for ib in range(IB2):
                    for h in range(HH):
                        i0 = ib * 1024
                        po = avp.tile([P, 1024], f32)
                        pd = denp.tile([1, 1024], f32)
                        ets = {}
                        etqs = {}
                        for jt in range(NJT + LAG):
                            if jt < NJT:
                                ps = dotsp.tile([P, 1024], f32, tag="ps")
                                k_l = kT_sb[:, h, jt * P:(jt + 1) * P]
                                for hf in range(2):
                                    nc.tensor.matmul(
                                        ps[:, hf * 512:(hf + 1) * 512],
                                        k_l,
                                        qT_sb[:, h,
                                              i0 + hf * 512:i0 + (hf + 1) * 512],
                                        start=True, stop=True)
                                et = expp.tile([P, 1024], bf16, tag="exp")
                                nc.scalar.activation(et, ps, Exp,
                                                     scale=SCALE_EXP)
                                ets[jt] = et
                            if jt >= LAG:
                                jd = jt - LAG
                                et = ets[jd]
                                v_l = v_sb[:, jd, h * P:(h + 1) * P]
                                for hf in range(2):
                                    sl = slice(hf * 512, (hf + 1) * 512)
                                    nc.tensor.matmul(
                                        po[:, sl], v_l, et[:, sl],
                                        start=(jd == 0), stop=(jd == NJT - 1))
                                if jd >= 12:
                                    for hf in range(2):
                                        sl = slice(hf * 512, (hf + 1) * 512)
                                        nc.tensor.matmul(
                                            pd[:, sl], ones_sb, et[:, sl],
                                            start=False, stop=(jd == NJT - 1))
                            if jt in (3, 7, 11):
                                q = jt // 4
                                s01 = sump.tile([P, 1024], bf16, tag="s01")
                                nc.vector.tensor_add(
                                    s01, ets[4 * q], ets[4 * q + 1])
                                s23 = sump.tile([P, 1024], bf16, tag="s23")
                                nc.vector.tensor_add(
                                    s23, ets[4 * q + 2], ets[4 * q + 3])
                                etq = sump.tile([P, 1024], bf16, tag="etq")
                                nc.vector.tensor_add(etq, s01, s23)
                                etqs[q] = etq
                            if jt in (6, 10, 14):
                                q = (jt - 6) // 4
                                etq = etqs.pop(q)
                                for hf in range(2):
                                    sl = slice(hf * 512, (hf + 1) * 512)
                                    nc.tensor.matmul(
                                        pd[:, sl], ones_sb, etq[:, sl],
                                        start=(q == 0), stop=False)
                        # Drain: free the PV and denominator PSUM banks as
                        # fast as possible (next block's accumulations wait
                        # on them), then normalize off the critical path.
                        osl = oT_ts[h][ib]
                        nc.vector.tensor_copy(osl, po)
                        dsb = drainp.tile([1, 1024], f32, tag="den")
                        nc.scalar.copy(dsb, pd)
                        rcf = drainp.tile([1, 1024], f32, tag="rcf")
                        nc.vector.reciprocal_approx_fast(rcf, dsb)
                        rcb = drainp.tile([1, 1024], bf16, tag="rcb")
                        nc.vector.tensor_copy(rcb, rcf)
                        bc = drainp.tile([P, 1024], bf16, tag="bc")
                        nc.gpsimd.partition_broadcast(bc, rcb)
                        nc.vector.tensor_mul(osl, osl, bc)

                # ---------------- phase 3: output projection ----------------
                # y-psum tiles share the dots pool slots (tag "ps"), which
                # free as the exp of the final j-tiles completes -- a fresh
                # PSUM pool would wait on the whole attention stack instead.
                with tc.tile_pool(name="wop", bufs=1) as wop, \
                     tc.tile_pool(name="ystage", bufs=4) as ystage:
                    wo_sb = wop.tile([P, HH, DIM], bf16)
                    nc.sync.dma_start(
                        out=wo_sb, in_=woT.rearrange("(dt p) e -> p dt e", p=P))
                    for ib in range(NIB):
                        bb, half = divmod(ib, 2)
                        for e8 in range(DIM // P):
                            ps = dotsp.tile([P, 512], f32, tag="ps")
                            for dt in range(HH):
                                nc.tensor.matmul(
                                    ps,
                                    wo_sb[:, dt, e8 * P:(e8 + 1) * P],
                                    oT_ts[dt][bb][:, half * 512:(half + 1) * 512],
                                    start=(dt == 0), stop=(dt == HH - 1))
                            ys = ystage.tile([P, 512], bf16, tag="ys")
                            if ib % 2 == 0:
                                nc.vector.tensor_copy(ys, ps)
                            else:
                                nc.scalar.copy(ys, ps)
                            nc.sync.dma_start(
                                out=yT[e8 * P:(e8 + 1) * P,
                                       ib * 512:(ib + 1) * 512],
                                in_=ys)

    nc.compile()
    return nc


_nc_by_reps = {}


def _get_nc(reps=1):
    if reps not in _nc_by_reps:
        _nc_by_reps[reps] = _build_nc(reps)
    return _nc_by_reps[reps]


def _make_in_maps(x_a, x_b, W_q, W_kv, W_out):
    from concourse import mybir
    BF = mybir.dt.np(mybir.dt.bfloat16)
    F8 = mybir.dt.np(mybir.dt.float8e4)
    xa8 = [np.ascontiguousarray(x_a[b].T).astype(F8) for b in range(B)]
    xb8 = [np.ascontiguousarray(x_b[b].T).astype(F8) for b in range(B)]
    xbT = [np.ascontiguousarray(x_b[b].T).astype(BF) for b in range(B)]
    in_maps = []
    for c in range(NCORES):
        b, hh = divmod(c, 2)
        hs = hh * DVC
        in_maps.append({
            "xa8": xa8[b],
            "xb8": xb8[b],
            "xbT": xbT[b],
            "wq8": (np.ascontiguousarray(W_q[hs:hs + DVC].T) * 16.0
                    ).astype(F8),
            "wk8": (np.ascontiguousarray(W_kv[hs:hs + DVC].T) * 16.0
                    ).astype(F8),
            "wvT": np.ascontiguousarray(
                W_kv[DIM + hs:DIM + hs + DVC].T).astype(BF),
            "woT": np.ascontiguousarray(W_out[:, hs:hs + DVC].T).astype(BF),
            "ones": np.ones((P, 1), dtype=BF),
        })
    return in_maps


def kernel(x_a, x_b, W_q, W_kv, W_out, b_out):
    global LAST_EXEC_NS, LAST_RESULTS
    from concourse import bass_utils

    x_a = np.asarray(x_a, dtype=np.float32)
    x_b = np.asarray(x_b, dtype=np.float32)
    W_q = np.asarray(W_q, dtype=np.float32)
    W_kv = np.asarray(W_kv, dtype=np.float32)
    W_out = np.asarray(W_out, dtype=np.float32)
    b_out = np.asarray(b_out, dtype=np.float32)

    nc = _get_nc(REPS)
    in_maps = _make_in_maps(x_a, x_b, W_q, W_kv, W_out)

    res = bass_utils.run_bass_kernel_spmd(
        nc, in_maps, core_ids=list(range(NCORES)), trace=_TRACE)
    LAST_EXEC_NS = res.exec_time_ns
    LAST_RESULTS = res

    out = np.empty((B, N, DIM), dtype=np.float32)
    for b in range(B):
        acc = (res.results[2 * b]["yT"].astype(np.float32)
               + res.results[2 * b + 1]["yT"].astype(np.float32))
        out[b] = acc.T + b_out
    return out


def bench(inputs, reps_pair=(1, 9), iters=5):
    """Measure on-device time per kernel body via rep-delta wall timing."""
    import time
    from concourse import bass_utils
    ins = {k: np.asarray(v, dtype=np.float32) for k, v in inputs.items()
           if k != "b_out"}
    in_maps = _make_in_maps(ins["x_a"], ins["x_b"], ins["W_q"], ins["W_kv"],
                            ins["W_out"])
    walls = {}
    for reps in reps_pair:
        nc = _get_nc(reps)
        # warm-up (compile+cache)
        bass_utils.run_bass_kernel_spmd(nc, in_maps, core_ids=list(range(NCORES)))
        ts = []
        for _ in range(iters):
            t0 = time.perf_counter()
            bass_utils.run_bass_kernel_spmd(nc, in_maps,
                                            core_ids=list(range(NCORES)))
            ts.append(time.perf_counter() - t0)
        walls[reps] = min(ts)
        print(f"reps={reps}: wall min={walls[reps]*1e3:.2f} ms  all={[f'{t*1e3:.1f}' for t in ts]}")
    r0, r1 = reps_pair
    ns = (walls[r1] - walls[r0]) / (r1 - r0) * 1e9
    print(f"per-body device time: {ns:.0f} ns")
    return ns
